# revision 12
# baseline (speedup 1.0000x reference)
"""Multi-head attention (B=2, S=2048, D=1024, H=16) on 8 trn2 NeuronCores.

Sharding: 8 cores = 2 batches x 4 head-groups (4 heads each).
Each core projects q/k/v for its 4 heads (256 of 1024 dims), computes
scores^T = k @ q^T per head, exp via ScalarE (no max-subtraction needed:
|scores| <~ 3), attn@V via TensorE with a ones-column in V producing the
softmax denominators for free, normalizes attention in-place (written to
DRAM transposed and in bf16 -- host fixes layout/dtype), and computes its
partial output projection. Host sums the 4 partials per batch + bias.

Matmuls and attention storage are bf16 (f32 PSUM accumulation); softmax
denominators/reciprocals stay fp32. Reciprocal rows are broadcast across
partitions via a DRAM round-trip (stride-0 partition reads are legal on
DRAM APs).
"""

import sys

if "/opt/trn_rl_repo" not in sys.path:
    sys.path.insert(0, "/opt/trn_rl_repo")

import ml_dtypes
import numpy as np

import concourse.bass as bass
import concourse.mybir as mybir
import concourse.tile as tile
from concourse import bacc
from concourse.bass_utils import run_bass_kernel_spmd

B, S, D, H = 2, 2048, 1024, 16
DH = D // H            # 64
NCORES = 8
HPC = H // 4           # heads per core: 4
DPC = HPC * DH         # head dims per core: 256
P = 128
KO = D // P            # 8 contraction chunks for the input projections
SO = S // P            # 16 s_k chunks of 128
SQC = 512              # s_q chunk width in phase B
NSQ = S // SQC         # 4 s_q chunks
SKB = 2                # s_k chunks per exp/DMA block
VW = DH + 2            # v cols per head: 64 + ones col + pad

F32 = mybir.dt.float32
BF16 = mybir.dt.bfloat16
AF = mybir.ActivationFunctionType
OP = mybir.AluOpType

NPBF16 = ml_dtypes.bfloat16


def _build_program():
    nc = bacc.Bacc("TRN2", target_bir_lowering=False, debug=False,
                   num_devices=NCORES)

    xq = nc.dram_tensor("xq_t", [D, S], BF16, kind="ExternalInput").ap()
    xk = nc.dram_tensor("xk_t", [D, S], BF16, kind="ExternalInput").ap()
    xv = nc.dram_tensor("xv_t", [D, S], BF16, kind="ExternalInput").ap()
    wq = nc.dram_tensor("wq_t", [D, DPC], BF16, kind="ExternalInput").ap()
    wk = nc.dram_tensor("wk_t", [D, DPC], BF16, kind="ExternalInput").ap()
    wv = nc.dram_tensor("wv_t", [D, DPC], BF16, kind="ExternalInput").ap()
    wo = nc.dram_tensor("wo_t", [DPC, D], BF16, kind="ExternalInput").ap()
    bq = nc.dram_tensor("bq_s", [DPC], F32, kind="ExternalInput").ap()
    bk = nc.dram_tensor("bk_s", [DPC], F32, kind="ExternalInput").ap()
    bv = nc.dram_tensor("bv_s", [1, DPC], BF16, kind="ExternalInput").ap()
    attn_t = nc.dram_tensor("attn_t", [HPC, S, S], BF16,
                            kind="ExternalOutput").ap()
    out_p = nc.dram_tensor("out_p", [S, D], F32, kind="ExternalOutput").ap()

    with tile.TileContext(nc) as tc:
        with (
            tc.tile_pool(name="persist", bufs=1) as wp,
            tc.tile_pool(name="xstream", bufs=4) as xp,
            tc.tile_pool(name="expp", bufs=3) as ep,
            tc.tile_pool(name="smalls", bufs=2) as sp,
            tc.tile_pool(name="outs", bufs=4) as op_,
            tc.tile_pool(name="ps_sc", bufs=2, space="PSUM") as ps_sc,
            tc.tile_pool(name="ps_av", bufs=2, space="PSUM") as ps_av,
            tc.tile_pool(name="ps_mm", bufs=2, space="PSUM") as ps_mm,
        ):
            # ---- persistent tiles -------------------------------------
            wq_sb = wp.tile([P, KO, DPC], BF16, tag="wq")
            wk_sb = wp.tile([P, KO, DPC], BF16, tag="wk")
            wv_sb = wp.tile([P, KO, DPC], BF16, tag="wv")
            wo_sb = wp.tile([P, DPC // P, D], BF16, tag="wo")
            bq_sb = wp.tile([P, DPC // P], F32, tag="bq")
            bk_sb = wp.tile([P, DPC // P], F32, tag="bk")
            bv1_sb = wp.tile([1, DPC], BF16, tag="bv1")
            ones1_sb = wp.tile([1, P], BF16, tag="ones1")
            qT = wp.tile([P, DPC // P, S], BF16, tag="qT")
            kT = wp.tile([P, DPC // P, S], BF16, tag="kT")
            v_aug = wp.tile([P, SO, HPC * VW], BF16, tag="vaug")
            avT = wp.tile([P, DPC // P, S], BF16, tag="avT")

            nc.sync.dma_start(wk_sb[:], wk.rearrange("(o i) m -> i o m", i=P))
            nc.sync.dma_start(wq_sb[:], wq.rearrange("(o i) m -> i o m", i=P))
            nc.sync.dma_start(wv_sb[:], wv.rearrange("(o i) m -> i o m", i=P))
            nc.sync.dma_start(wo_sb[:], wo.rearrange("(o i) n -> i o n", i=P))
            nc.sync.dma_start(bk_sb[:], bk.rearrange("(c p) -> p c", p=P))
            nc.sync.dma_start(bq_sb[:], bq.rearrange("(c p) -> p c", p=P))
            nc.sync.dma_start(bv1_sb[:], bv[:])
            nc.vector.memset(ones1_sb[:], 1.0)
            # ones (+pad) columns of v_aug
            nc.vector.memset(
                v_aug[:].rearrange("p o (h c) -> p o h c", c=VW)[:, :, :, DH:],
                1.0,
            )

            # ---- phase A: projections ---------------------------------
            # qT/kT[p, c, s] = (x @ W.T + b).T for this core's 256 dims
            for xin, w_sb, b_sb, dst in (
                (xk, wk_sb, bk_sb, kT),
                (xq, wq_sb, bq_sb, qT),
            ):
                xr = xin.rearrange("(o i) s -> i o s", i=P)
                for j in range(S // 1024):
                    xt = xp.tile([P, KO, 1024], BF16, tag="xt")
                    nc.sync.dma_start(xt[:],
                                      xr[:, :, j * 1024:(j + 1) * 1024])
                    for jj in range(2):
                        s0 = j * 1024 + jj * 512
                        for c in range(DPC // P):
                            pp = ps_mm.tile([P, 512], F32, tag="mm")
                            for k in range(KO):
                                nc.tensor.matmul(
                                    pp[:],
                                    w_sb[:, k, c * P:(c + 1) * P],
                                    xt[:, k, jj * 512:(jj + 1) * 512],
                                    start=(k == 0), stop=(k == KO - 1),
                                )
                            nc.scalar.activation(
                                dst[:, c, s0:s0 + 512], pp[:],
                                AF.Identity, bias=b_sb[:, c:c + 1],
                            )
            # v in natural [s, dv] layout, interleaved with ones columns
            xr = xv.rearrange("(o i) s -> i o s", i=P)
            for j2 in range(S // 1024):
                xt = xp.tile([P, KO, 1024], BF16, tag="xt")
                nc.sync.dma_start(xt[:],
                                  xr[:, :, j2 * 1024:(j2 + 1) * 1024])
              # keep 512-granular inner structure
                for cc in range(8):
                    pp = ps_mm.tile([P, 512], F32, tag="mm")
                    for k in range(KO):
                        nc.tensor.matmul(
                            pp[:, :DPC],
                            xt[:, k, cc * P:(cc + 1) * P],
                            wv_sb[:, k, :],
                            start=(k == 0), stop=False,
                        )
                    # bias via K=1 ones x bv accumulation
                    nc.tensor.matmul(
                        pp[:, :DPC], ones1_sb[:], bv1_sb[:],
                        start=False, stop=True,
                    )
                    so = j2 * 8 + cc
                    for h in range(HPC):
                        nc.scalar.copy(
                            v_aug[:, so, h * VW:h * VW + DH],
                            pp[:, h * DH:(h + 1) * DH],
                        )

            # ---- phase B: attention, software-pipelined across heads ----
            # For each (j, h): emit scores+exp for (h) interleaved with the
            # attn@V matmuls of the PREVIOUS head, so ScalarE (exp) never
            # waits on PE av-bursts and phase A's v-projection hides under
            # the first exps.
            def phase_c(j):
                # output projection for s-rows [j*SQC, (j+1)*SQC)
                for m in range(j * (SQC // P), (j + 1) * (SQC // P)):
                    for n in range(D // 512):
                        pp = ps_mm.tile([P, 512], F32, tag="mm")
                        for k in range(DPC // P):
                            nc.tensor.matmul(
                                pp[:],
                                avT[:, k, m * P:(m + 1) * P],
                                wo_sb[:, k, n * 512:(n + 1) * 512],
                                start=(k == 0), stop=(k == DPC // P - 1),
                            )
                        osb = op_.tile([P, 512], F32, tag="osb")
                        nc.scalar.copy(osb[:], pp[:])
                        nc.sync.dma_start(
                            out_p[m * P:(m + 1) * P, n * 512:(n + 1) * 512],
                            osb[:],
                        )

            def av_tail(ph, pj, pexpt, pavp):
                # reciprocal of softmax sums, normalize + store attention,
                # and the normalized-av columns for the output projection
                ppb = (ph % 2) * DH
                pc = ph // 2
                psq = slice(pj * SQC, (pj + 1) * SQC)
                rec1 = sp.tile([1, SQC], BF16, tag="rec1")
                with nc.allow_low_precision(reason="softmax recip bf16"):
                    nc.vector.reciprocal(rec1[:], pavp[DH:DH + 1, :])
                recip = sp.tile([P, SQC], BF16, tag="recip")
                nc.gpsimd.partition_broadcast(recip[:], rec1[:])
                att_dst = attn_t[ph].rearrange("(o i) q -> i o q", i=P)
                for grp in range(SO // (2 * SKB)):
                    bs = slice(grp * 2 * SKB, (grp + 1) * 2 * SKB)
                    for t in range(2 * SKB):
                        sk = grp * 2 * SKB + t
                        nc.vector.tensor_tensor(
                            pexpt[:, sk, :], pexpt[:, sk, :], recip[:],
                            OP.mult,
                        )
                    nc.sync.dma_start(att_dst[:, bs, psq], pexpt[:, bs, :])
                nc.vector.tensor_tensor(
                    avT[ppb:ppb + DH, pc, psq], pavp[:DH, :], recip[:DH, :],
                    OP.mult,
                )

            prev = None        # (h, j, expt, avp) with av matmuls pending
            for j in range(NSQ):
                sq = slice(j * SQC, (j + 1) * SQC)
                for h in range(HPC):
                    pb = (h % 2) * DH
                    c = h // 2
                    expt = ep.tile([P, SO, SQC], BF16, tag="expT")
                    avp = ps_av.tile([P, SQC], F32, tag="av")
                    for blk in range(SO // SKB):
                        scp = ps_sc.tile([P, SKB, SQC], F32, tag="sc")
                        for t in range(SKB):
                            sk = blk * SKB + t
                            nc.tensor.matmul(
                                scp[:, t, :],
                                kT[pb:pb + DH, c, sk * P:(sk + 1) * P],
                                qT[pb:pb + DH, c, sq],
                                start=True, stop=True,
                            )
                        nc.scalar.activation(
                            expt[:, blk * SKB:(blk + 1) * SKB, :], scp[:],
                            AF.Exp, scale=float(1.0 / np.sqrt(DH)),
                        )
                        if prev is not None:
                            ph, pj, pexpt, pavp = prev
                            for t in range(SKB):
                                sk = blk * SKB + t
                                nc.tensor.matmul(
                                    pavp[:VW, :],
                                    v_aug[:, sk, ph * VW:(ph + 1) * VW],
                                    pexpt[:, sk, :],
                                    start=(sk == 0), stop=(sk == SO - 1),
                                )
                        # filler matmul: keeps the PE activity monitor warm
                        # (an idle-ish PE re-throttles to 1.2 GHz, doubling
                        # every matmul)
                        fil = ps_mm.tile([P, 512], F32, tag="mm")
                        nc.tensor.matmul(
                            fil[:, :256], kT[:, 0, :P], qT[:, 0, :256],
                            start=True, stop=True,
                        )
                    if prev is not None:
                        av_tail(*prev)
                    prev = (h, j, expt, avp)
                if j > 1:
                    phase_c(j - 2)
            # drain the last head's av matmuls
            ph, pj, pexpt, pavp = prev
            for sk in range(SO):
                nc.tensor.matmul(
                    pavp[:VW, :],
                    v_aug[:, sk, ph * VW:(ph + 1) * VW],
                    pexpt[:, sk, :],
                    start=(sk == 0), stop=(sk == SO - 1),
                )
            av_tail(*prev)
            phase_c(NSQ - 2)
            phase_c(NSQ - 1)

    nc.compile()
    return nc


_NC = None


def _get_program():
    global _NC
    if _NC is None:
        _NC = _build_program()
    return _NC


def kernel(query, key, value, Wq, bq, Wk, bk, Wv, bv, Wo, bo, *, trace=False):
    query = np.asarray(query, np.float32)
    key = np.asarray(key, np.float32)
    value = np.asarray(value, np.float32)
    Wq, bq = np.asarray(Wq, np.float32), np.asarray(bq, np.float32)
    Wk, bk = np.asarray(Wk, np.float32), np.asarray(bk, np.float32)
    Wv, bv = np.asarray(Wv, np.float32), np.asarray(bv, np.float32)
    Wo, bo = np.asarray(Wo, np.float32), np.asarray(bo, np.float32)

    nc = _get_program()

    in_maps = []
    xt = [np.ascontiguousarray(x.T).astype(NPBF16)
          for x in (*query, *key, *value)]  # per batch, [D, S] bf16
    WqT, WkT, WvT, WoT = Wq.T, Wk.T, Wv.T, Wo.T
    for core in range(NCORES):
        b, hg = divmod(core, 4)
        sl = slice(hg * DPC, (hg + 1) * DPC)
        in_maps.append({
            "xq_t": xt[b],
            "xk_t": xt[B + b],
            "xv_t": xt[2 * B + b],
            "wq_t": np.ascontiguousarray(WqT[:, sl]).astype(NPBF16),
            "wk_t": np.ascontiguousarray(WkT[:, sl]).astype(NPBF16),
            "wv_t": np.ascontiguousarray(WvT[:, sl]).astype(NPBF16),
            "wo_t": np.ascontiguousarray(WoT[sl, :]).astype(NPBF16),
            "bq_s": np.ascontiguousarray(bq[sl]),
            "bk_s": np.ascontiguousarray(bk[sl]),
            "bv_s": np.ascontiguousarray(bv[sl]).astype(NPBF16)[None, :],
        })

    res = run_bass_kernel_spmd(nc, in_maps, core_ids=list(range(NCORES)),
                               trace=trace)

    out = np.broadcast_to(bo, (B, S, D)).copy()
    attn = np.empty((B, H, S, S), np.float32)
    for core in range(NCORES):
        b, hg = divmod(core, 4)
        out[b] += res.results[core]["out_p"]
        at = res.results[core]["attn_t"]  # [HPC, s_k, s_q] bf16
        for h in range(HPC):
            attn[b, hg * HPC + h] = at[h].T
    if trace:
        kernel.last_exec_time_ns = res.exec_time_ns
        kernel.last_results = res
    return out, attn


# revision 13
# speedup vs baseline: 1.0241x; 1.0241x over previous
"""Multi-head attention (B=2, S=2048, D=1024, H=16) on 8 trn2 NeuronCores.

Sharding: 8 cores = 2 batches x 4 head-groups (4 heads each).
Each core projects q/k/v for its 4 heads (256 of 1024 dims), computes
scores^T = k @ q^T per head, exp via ScalarE (no max-subtraction needed:
|scores| <~ 3), attn@V via TensorE with a ones-column in V producing the
softmax denominators for free, normalizes attention in-place (written to
DRAM transposed and in bf16 -- host fixes layout/dtype), and computes its
partial output projection. Host sums the 4 partials per batch + bias.

Matmuls and attention storage are bf16 (f32 PSUM accumulation); softmax
denominators/reciprocals stay fp32. Reciprocal rows are broadcast across
partitions via a DRAM round-trip (stride-0 partition reads are legal on
DRAM APs).
"""

import sys

if "/opt/trn_rl_repo" not in sys.path:
    sys.path.insert(0, "/opt/trn_rl_repo")

import ml_dtypes
import numpy as np

import concourse.bass as bass
import concourse.mybir as mybir
import concourse.tile as tile
from concourse import bacc
from concourse.bass_utils import run_bass_kernel_spmd

B, S, D, H = 2, 2048, 1024, 16
DH = D // H            # 64
NCORES = 8
HPC = H // 4           # heads per core: 4
DPC = HPC * DH         # head dims per core: 256
P = 128
KO = D // P            # 8 contraction chunks for the input projections
SO = S // P            # 16 s_k chunks of 128
SQC = 512              # s_q chunk width in phase B
NSQ = S // SQC         # 4 s_q chunks
SKB = 2                # s_k chunks per exp/DMA block
VW = DH + 2            # v cols per head: 64 + ones col + pad

F32 = mybir.dt.float32
BF16 = mybir.dt.bfloat16
AF = mybir.ActivationFunctionType
OP = mybir.AluOpType

NPBF16 = ml_dtypes.bfloat16


def _build_program():
    nc = bacc.Bacc("TRN2", target_bir_lowering=False, debug=False,
                   num_devices=NCORES)

    xq = nc.dram_tensor("xq_t", [D, S], BF16, kind="ExternalInput").ap()
    xk = nc.dram_tensor("xk_t", [D, S], BF16, kind="ExternalInput").ap()
    xv = nc.dram_tensor("xv_t", [D, S], BF16, kind="ExternalInput").ap()
    wq = nc.dram_tensor("wq_t", [D, DPC], BF16, kind="ExternalInput").ap()
    wk = nc.dram_tensor("wk_t", [D, DPC], BF16, kind="ExternalInput").ap()
    wv = nc.dram_tensor("wv_t", [D, DPC], BF16, kind="ExternalInput").ap()
    wo = nc.dram_tensor("wo_t", [DPC, D], BF16, kind="ExternalInput").ap()
    bq = nc.dram_tensor("bq_s", [DPC], F32, kind="ExternalInput").ap()
    bk = nc.dram_tensor("bk_s", [DPC], F32, kind="ExternalInput").ap()
    bv = nc.dram_tensor("bv_s", [1, DPC], BF16, kind="ExternalInput").ap()
    attn_t = nc.dram_tensor("attn_t", [HPC, S, S], BF16,
                            kind="ExternalOutput").ap()
    out_p = nc.dram_tensor("out_p", [S, D], F32, kind="ExternalOutput").ap()

    with tile.TileContext(nc) as tc:
        with (
            tc.tile_pool(name="persist", bufs=1) as wp,
            tc.tile_pool(name="xstream", bufs=4) as xp,
            tc.tile_pool(name="expp", bufs=4) as ep,
            tc.tile_pool(name="smalls", bufs=2) as sp,
            tc.tile_pool(name="outs", bufs=4) as op_,
            tc.tile_pool(name="ps_sc", bufs=2, space="PSUM") as ps_sc,
            tc.tile_pool(name="ps_av", bufs=2, space="PSUM") as ps_av,
            tc.tile_pool(name="ps_mm", bufs=2, space="PSUM") as ps_mm,
        ):
            # ---- persistent tiles -------------------------------------
            wq_sb = wp.tile([P, KO, DPC], BF16, tag="wq")
            wk_sb = wp.tile([P, KO, DPC], BF16, tag="wk")
            wv_sb = wp.tile([P, KO, DPC], BF16, tag="wv")
            wo_sb = wp.tile([P, DPC // P, D], BF16, tag="wo")
            bq_sb = wp.tile([P, DPC // P], F32, tag="bq")
            bk_sb = wp.tile([P, DPC // P], F32, tag="bk")
            bv1_sb = wp.tile([1, DPC], BF16, tag="bv1")
            ones1_sb = wp.tile([1, P], BF16, tag="ones1")
            qT = wp.tile([P, DPC // P, S], BF16, tag="qT")
            kT = wp.tile([P, DPC // P, S], BF16, tag="kT")
            v_aug = wp.tile([P, SO, HPC * VW], BF16, tag="vaug")
            avT = wp.tile([P, DPC // P, S], BF16, tag="avT")

            nc.sync.dma_start(wk_sb[:], wk.rearrange("(o i) m -> i o m", i=P))
            nc.sync.dma_start(wq_sb[:], wq.rearrange("(o i) m -> i o m", i=P))
            nc.sync.dma_start(wv_sb[:], wv.rearrange("(o i) m -> i o m", i=P))
            nc.sync.dma_start(wo_sb[:], wo.rearrange("(o i) n -> i o n", i=P))
            nc.sync.dma_start(bk_sb[:], bk.rearrange("(c p) -> p c", p=P))
            nc.sync.dma_start(bq_sb[:], bq.rearrange("(c p) -> p c", p=P))
            nc.sync.dma_start(bv1_sb[:], bv[:])
            nc.vector.memset(ones1_sb[:], 1.0)
            # ones (+pad) columns of v_aug
            nc.vector.memset(
                v_aug[:].rearrange("p o (h c) -> p o h c", c=VW)[:, :, :, DH:],
                1.0,
            )

            # ---- phase A pieces ---------------------------------------
            def project_qk(xin, w_sb, b_sb, dst):
                # dst[p, c, s] = (x @ W.T + b).T for this core's 256 dims
                xr = xin.rearrange("(o i) s -> i o s", i=P)
                for j in range(S // 512):
                    xt = xp.tile([P, KO, 512], BF16, tag="xt")
                    nc.sync.dma_start(xt[:], xr[:, :, j * 512:(j + 1) * 512])
                    for c in range(DPC // P):
                        pp = ps_mm.tile([P, 512], F32, tag="mm")
                        for k in range(KO):
                            nc.tensor.matmul(
                                pp[:],
                                w_sb[:, k, c * P:(c + 1) * P],
                                xt[:, k, :],
                                start=(k == 0), stop=(k == KO - 1),
                            )
                        nc.scalar.activation(
                            dst[:, c, j * 512:(j + 1) * 512], pp[:],
                            AF.Identity, bias=b_sb[:, c:c + 1],
                        )

            def project_v():
                # v in natural [s, dv] layout, ones columns interleaved,
                # bias folded in via a K=1 ones x bv matmul
                xr = xv.rearrange("(o i) s -> i o s", i=P)
                for j in range(S // 512):
                    xt = xp.tile([P, KO, 512], BF16, tag="xt")
                    nc.sync.dma_start(xt[:], xr[:, :, j * 512:(j + 1) * 512])
                    for c in range(4):
                        pp = ps_mm.tile([P, 512], F32, tag="mm")
                        for k in range(KO):
                            nc.tensor.matmul(
                                pp[:, :DPC],
                                xt[:, k, c * P:(c + 1) * P],
                                wv_sb[:, k, :],
                                start=(k == 0), stop=False,
                            )
                        nc.tensor.matmul(
                            pp[:, :DPC], ones1_sb[:], bv1_sb[:],
                            start=False, stop=True,
                        )
                        so = j * 4 + c
                        for h in range(HPC):
                            nc.vector.tensor_copy(
                                v_aug[:, so, h * VW:h * VW + DH],
                                pp[:, h * DH:(h + 1) * DH],
                            )

            def phase_c(j):
                # output projection for s-rows [j*SQC, (j+1)*SQC)
                for m in range(j * (SQC // P), (j + 1) * (SQC // P)):
                    for n in range(D // 512):
                        pp = ps_mm.tile([P, 512], F32, tag="mm")
                        for k in range(DPC // P):
                            nc.tensor.matmul(
                                pp[:],
                                avT[:, k, m * P:(m + 1) * P],
                                wo_sb[:, k, n * 512:(n + 1) * 512],
                                start=(k == 0), stop=(k == DPC // P - 1),
                            )
                        osb = op_.tile([P, 512], F32, tag="osb")
                        nc.scalar.copy(osb[:], pp[:])
                        nc.sync.dma_start(
                            out_p[m * P:(m + 1) * P, n * 512:(n + 1) * 512],
                            osb[:],
                        )

            def av_tail(ph, pj, pexpt, pavp):
                # softmax reciprocal, normalize + store attention, and the
                # normalized-av columns for the output projection
                ppb = (ph % 2) * DH
                pc = ph // 2
                psq = slice(pj * SQC, (pj + 1) * SQC)
                rec1 = sp.tile([1, SQC], BF16, tag="rec1")
                with nc.allow_low_precision(reason="softmax recip bf16"):
                    nc.vector.reciprocal(rec1[:], pavp[DH:DH + 1, :])
                recip = sp.tile([P, SQC], BF16, tag="recip")
                nc.gpsimd.partition_broadcast(recip[:], rec1[:])
                att_dst = attn_t[ph].rearrange("(o i) q -> i o q", i=P)
                for grp in range(SO // (2 * SKB)):
                    bs = slice(grp * 2 * SKB, (grp + 1) * 2 * SKB)
                    for t in range(2 * SKB):
                        sk = grp * 2 * SKB + t
                        nc.vector.tensor_tensor(
                            pexpt[:, sk, :], pexpt[:, sk, :], recip[:],
                            OP.mult,
                        )
                    nc.sync.dma_start(att_dst[:, bs, psq], pexpt[:, bs, :])
                nc.vector.tensor_tensor(
                    avT[ppb:ppb + DH, pc, psq], pavp[:DH, :], recip[:DH, :],
                    OP.mult,
                )

            # ---- phases A+B+C, software-pipelined ---------------------
            # Emission: k/q projections; scores+exp sections for (j, h)
            # run LAG=2 sections ahead of their attn@V matmuls, so the
            # v-projection and every av burst hide under the exp stream.
            project_qk(xk, wk_sb, bk_sb, kT)
            project_qk(xq, wq_sb, bq_sb, qT)

            LAG = 2
            sections = [(j, h) for j in range(NSQ) for h in range(HPC)]
            pending = []

            def do_av_section(front_h, front_j, front_expt):
                avp = ps_av.tile([P, SQC], F32, tag="av")
                return (front_h, front_j, front_expt, avp)

            for g, (j, h) in enumerate(sections):
                if g == LAG:
                    project_v()
                pb = (h % 2) * DH
                c = h // 2
                sq = slice(j * SQC, (j + 1) * SQC)
                expt = ep.tile([P, SO, SQC], BF16, tag="expT")
                front = None
                if len(pending) >= LAG:
                    fh, fj, fexpt = pending.pop(0)
                    front = do_av_section(fh, fj, fexpt)
                for blk in range(SO // SKB):
                    scp = ps_sc.tile([P, SKB, SQC], F32, tag="sc")
                    for t in range(SKB):
                        sk = blk * SKB + t
                        nc.tensor.matmul(
                            scp[:, t, :],
                            kT[pb:pb + DH, c, sk * P:(sk + 1) * P],
                            qT[pb:pb + DH, c, sq],
                            start=True, stop=True,
                        )
                    nc.scalar.activation(
                        expt[:, blk * SKB:(blk + 1) * SKB, :], scp[:],
                        AF.Exp, scale=float(1.0 / np.sqrt(DH)),
                    )
                    if front is not None:
                        fh, fj, fexpt, favp = front
                        for t in range(SKB):
                            sk = blk * SKB + t
                            nc.tensor.matmul(
                                favp[:VW, :],
                                v_aug[:, sk, fh * VW:(fh + 1) * VW],
                                fexpt[:, sk, :],
                                start=(sk == 0), stop=(sk == SO - 1),
                            )
                    # filler matmul: keeps the PE activity monitor warm (an
                    # idle-ish PE re-throttles to 1.2 GHz, doubling matmuls)
                    fil = ps_mm.tile([P, 512], F32, tag="mm")
                    nc.tensor.matmul(
                        fil[:, :256], kT[:, 0, :P], qT[:, 0, :256],
                        start=True, stop=True,
                    )
                if front is not None:
                    av_tail(*front)
                pending.append((h, j, expt))
                if h == HPC - 1 and j >= LAG:
                    phase_c(j - LAG)

            # drain the pipeline
            for fh, fj, fexpt in pending:
                favp = ps_av.tile([P, SQC], F32, tag="av")
                for sk in range(SO):
                    nc.tensor.matmul(
                        favp[:VW, :],
                        v_aug[:, sk, fh * VW:(fh + 1) * VW],
                        fexpt[:, sk, :],
                        start=(sk == 0), stop=(sk == SO - 1),
                    )
                av_tail(fh, fj, fexpt, favp)
            for j in range(NSQ - LAG, NSQ):
                phase_c(j)

    nc.compile()
    return nc


_NC = None


def _get_program():
    global _NC
    if _NC is None:
        _NC = _build_program()
    return _NC


def kernel(query, key, value, Wq, bq, Wk, bk, Wv, bv, Wo, bo, *, trace=False):
    query = np.asarray(query, np.float32)
    key = np.asarray(key, np.float32)
    value = np.asarray(value, np.float32)
    Wq, bq = np.asarray(Wq, np.float32), np.asarray(bq, np.float32)
    Wk, bk = np.asarray(Wk, np.float32), np.asarray(bk, np.float32)
    Wv, bv = np.asarray(Wv, np.float32), np.asarray(bv, np.float32)
    Wo, bo = np.asarray(Wo, np.float32), np.asarray(bo, np.float32)

    nc = _get_program()

    in_maps = []
    xt = [np.ascontiguousarray(x.T).astype(NPBF16)
          for x in (*query, *key, *value)]  # per batch, [D, S] bf16
    WqT, WkT, WvT, WoT = Wq.T, Wk.T, Wv.T, Wo.T
    for core in range(NCORES):
        b, hg = divmod(core, 4)
        sl = slice(hg * DPC, (hg + 1) * DPC)
        in_maps.append({
            "xq_t": xt[b],
            "xk_t": xt[B + b],
            "xv_t": xt[2 * B + b],
            "wq_t": np.ascontiguousarray(WqT[:, sl]).astype(NPBF16),
            "wk_t": np.ascontiguousarray(WkT[:, sl]).astype(NPBF16),
            "wv_t": np.ascontiguousarray(WvT[:, sl]).astype(NPBF16),
            "wo_t": np.ascontiguousarray(WoT[sl, :]).astype(NPBF16),
            "bq_s": np.ascontiguousarray(bq[sl]),
            "bk_s": np.ascontiguousarray(bk[sl]),
            "bv_s": np.ascontiguousarray(bv[sl]).astype(NPBF16)[None, :],
        })

    res = run_bass_kernel_spmd(nc, in_maps, core_ids=list(range(NCORES)),
                               trace=trace)

    out = np.broadcast_to(bo, (B, S, D)).copy()
    attn = np.empty((B, H, S, S), np.float32)
    for core in range(NCORES):
        b, hg = divmod(core, 4)
        out[b] += res.results[core]["out_p"]
        at = res.results[core]["attn_t"]  # [HPC, s_k, s_q] bf16
        for h in range(HPC):
            attn[b, hg * HPC + h] = at[h].T
    if trace:
        kernel.last_exec_time_ns = res.exec_time_ns
        kernel.last_results = res
    return out, attn
